# revision 3
# baseline (speedup 1.0000x reference)
"""Multi-head causal attention (B=4, S=2048, D=1024, H=16, hd=64) on 8 trn2 cores.

Sharding: core i handles batch b=i//2 and head-group hg=i%2 (8 heads).
Each core computes partial_out_b = ctx(heads of hg) @ Wo[rows of hg].
Host: out[b] = partial[2b] + partial[2b+1] + bo.

Per-core kernel (all matmuls bf16 with fp32 psum accumulation):
  A) x [2048,1024] f32 -> PE-transpose -> xT [128,8dc,2048] bf16
  B) weights load+cast; v = x@Wv_h -> [128,16st,8h,65] bf16 (col 64 = ones,
     so the attention matmul also produces softmax denominators);
     qT/kT = (x@Wq_h).T layout [128(2 heads),4g,2048] bf16
  C) per head h, q-block j (512 q): for t-chunk c: sT=k@qT scores (causal-
     trimmed), exp on ScalarE (scale=1/8, no max-sub: |scores/8|<~3),
     upper-tri mask on the diagonal 128x128 block, ctx accumulation
     ctxT[65,512] += v_aug.T @ p. Row 64 = denominator. Normalize via
     K=1 ones-matmul broadcast + reciprocal_approx_fast + mul -> ctxT bf16.
  D) out = ctxT.T @ Wo_h -> [2048,1024] f32 partial output.
"""
import os
import sys

for _p in ("/opt/trn_rl_repo",):
    if os.path.isdir(_p) and _p not in sys.path:
        sys.path.insert(0, _p)

import numpy as np
from contextlib import ExitStack

import concourse.bass as bass
from concourse import bacc
import concourse.mybir as mybir
import concourse.tile as tile
from concourse import bass_utils
from concourse.masks import make_upper_triangular, make_identity

F32 = mybir.dt.float32
BF16 = mybir.dt.bfloat16
EXP = mybir.ActivationFunctionType.Exp

S = 2048          # sequence length
D = 1024          # d_model
P = 128           # partitions
NT = S // P       # 16 s-tiles
DC = D // P       # 8 d-model chunks
NH = 8            # heads per core
HD = 64           # head dim
NG = NH // 2      # head pairs (lhsT col groups of 128)
NJ = S // 512     # 4 q-blocks of 512
SCALE = HD ** -0.5

_CACHED_NC = None


def build_nc():
    nc = bacc.Bacc("TRN2", target_bir_lowering=False)
    x_d = nc.dram_tensor("x", (S, D), F32, kind="ExternalInput")
    wq_d = nc.dram_tensor("wq", (D, 512), F32, kind="ExternalInput")
    wk_d = nc.dram_tensor("wk", (D, 512), F32, kind="ExternalInput")
    wv_d = nc.dram_tensor("wv", (D, 512), F32, kind="ExternalInput")
    wo_d = nc.dram_tensor("wo", (512, D), F32, kind="ExternalInput")
    out_d = nc.dram_tensor("part", (S, D), F32, kind="ExternalOutput")

    with tile.TileContext(nc) as tc, ExitStack() as ctx:
        persist = ctx.enter_context(tc.tile_pool(name="persist", bufs=1))
        stage = ctx.enter_context(tc.tile_pool(name="stage", bufs=2))
        work = ctx.enter_context(tc.tile_pool(name="work", bufs=3))
        pT_pool = ctx.enter_context(tc.tile_pool(name="pT", bufs=4))
        norm_pool = ctx.enter_context(tc.tile_pool(name="norm", bufs=2))
        # PSUM budget: ab(2) + s(3) + ctx(2) + bc(1) = 8 banks
        ps_ab = ctx.enter_context(tc.tile_pool(name="ps_ab", bufs=2, space="PSUM"))
        ps_s = ctx.enter_context(tc.tile_pool(name="ps_s", bufs=3, space="PSUM"))
        ps_ctx = ctx.enter_context(tc.tile_pool(name="ps_ctx", bufs=2, space="PSUM"))
        ps_bc = ctx.enter_context(tc.tile_pool(name="ps_bc", bufs=1, space="PSUM"))

        # --- constants ---
        ident = persist.tile([P, P], F32)
        make_identity(nc, ident)
        tri = persist.tile([P, P], BF16)           # upper-tri incl diag (t<=q valid)
        make_upper_triangular(nc, tri, val=1.0, diag=True)
        ones1 = persist.tile([P, HD], F32)   # row 64 used as K=1 lhsT (base par 64)
        nc.vector.memset(ones1, 1.0)

        # --- persistent tensors ---
        xT = persist.tile([P, DC, S], BF16)        # [p, dc, s] : xT[dc*128+p, s]
        qT = persist.tile([P, NG, S], BF16)        # [p, g, s]  : row p = head-pair col
        kT = persist.tile([P, NG, S], BF16)
        v_all = persist.tile([P, NT, NH, HD + 1], BF16)
        ctxT = persist.tile([P, NG, S], BF16)

        # --- phase A: load x, transpose to xT ---
        for st in range(NT):
            x_f = work.tile([P, D], F32, tag="x_f")
            nc.sync.dma_start(x_f, x_d[st * P:(st + 1) * P, :])
            for dc in range(DC):
                tp = ps_ab.tile([P, 512], F32, tag="ab")
                nc.tensor.transpose(tp[:, 0:P], x_f[:, dc * P:(dc + 1) * P], ident)
                nc.vector.tensor_copy(xT[:, dc, st * P:(st + 1) * P], tp[:, 0:P])

        # --- phase B: weights ---
        wq_f = stage.tile([P, DC, 512], F32, tag="w_f")
        nc.sync.dma_start(wq_f, wq_d.rearrange("(dc p) c -> p dc c", p=P))
        wq_b = persist.tile([P, DC, 512], BF16)
        nc.vector.tensor_copy(wq_b, wq_f)
        wk_f = stage.tile([P, DC, 512], F32, tag="w_f")
        nc.sync.dma_start(wk_f, wk_d.rearrange("(dc p) c -> p dc c", p=P))
        wk_b = persist.tile([P, DC, 512], BF16)
        nc.vector.tensor_copy(wk_b, wk_f)
        wv_f = stage.tile([P, DC, 512], F32, tag="w_f")
        nc.sync.dma_start(wv_f, wv_d.rearrange("(dc p) c -> p dc c", p=P))
        wv_b = persist.tile([P, DC, 512], BF16)
        nc.vector.tensor_copy(wv_b, wv_f)
        wo_f = stage.tile([P, 4, D], F32, tag="w_f")
        nc.sync.dma_start(wo_f, wo_d.rearrange("(g p) e -> p g e", p=P))
        wo_b = persist.tile([P, 4, D], BF16)
        nc.vector.tensor_copy(wo_b, wo_f)

        # v = x @ Wv_h  (natural layout, strided by 65 with ones col)
        nc.vector.memset(v_all[:, :, :, HD], 1.0)
        for st in range(NT):
            pv = ps_ab.tile([P, 512], F32, tag="ab")
            for dc in range(DC):
                nc.tensor.matmul(pv, xT[:, dc, st * P:(st + 1) * P], wv_b[:, dc, :],
                                 start=(dc == 0), stop=(dc == DC - 1))
            nc.vector.tensor_copy(
                v_all[:, st, :, 0:HD],
                pv.rearrange("p (h e) -> p h e", h=NH))

        # --- phases B2+C interleaved per head-pair g ---
        for g in range(NG):
            # qT_g, kT_g projections
            for w_b, dstT in ((wq_b, qT), (wk_b, kT)):
                for sb in range(NJ):
                    pq = ps_ab.tile([P, 512], F32, tag="ab")
                    for dc in range(DC):
                        nc.tensor.matmul(
                            pq, w_b[:, dc, g * P:(g + 1) * P],
                            xT[:, dc, sb * 512:(sb + 1) * 512],
                            start=(dc == 0), stop=(dc == DC - 1))
                    nc.vector.tensor_copy(dstT[:, g, sb * 512:(sb + 1) * 512], pq)

            # attention for the two heads of pair g
            for hh in range(2):
                h = 2 * g + hh
                row = hh * HD
                for j in range(NJ):
                    ctx_ps = ps_ctx.tile([HD + 1, 512], F32, tag="ctx")
                    nchunks = 4 * j + 4
                    for c in range(nchunks):
                        r = c - 4 * j          # >=0 for diagonal chunks
                        n0 = 128 * r if r >= 0 else 0
                        nn = 512 - n0
                        s_ps = ps_s.tile([P, 512], F32, tag="s")
                        nc.tensor.matmul(
                            s_ps[:, n0:512],
                            kT[row:row + HD, g, c * P:(c + 1) * P],
                            qT[row:row + HD, g, j * 512 + n0:(j + 1) * 512],
                            start=True, stop=True)
                        pT = pT_pool.tile([P, 512], BF16, tag="pT")
                        nc.scalar.activation(pT[:, n0:512], s_ps[:, n0:512],
                                             EXP, scale=SCALE)
                        if r >= 0:
                            nc.vector.tensor_mul(pT[:, n0:n0 + P],
                                                 pT[:, n0:n0 + P], tri)
                        nc.tensor.matmul(
                            ctx_ps[:, n0:512], v_all[:, c, h, :], pT[:, n0:512],
                            start=(c == 0), stop=(c == nchunks - 1))
                    # normalization
                    raw = norm_pool.tile([HD + 1, 512], F32, tag="raw")
                    nc.vector.tensor_copy(raw, ctx_ps)
                    bc = ps_bc.tile([HD, 512], F32, tag="bc")
                    nc.tensor.matmul(bc, ones1[HD:HD + 1, :], raw[HD:HD + 1, :],
                                     start=True, stop=True)
                    rec = norm_pool.tile([HD, 512], F32, tag="rec")
                    nc.vector.reciprocal_approx_fast(rec, bc)
                    nc.vector.tensor_tensor(
                        ctxT[row:row + HD, g, j * 512:(j + 1) * 512],
                        raw[0:HD, :], rec, mybir.AluOpType.mult)

        # --- phase D: out projection ---
        for st in range(NT):
            for eb in range(2):
                po = ps_ab.tile([P, 512], F32, tag="ab")
                for g in range(NG):
                    nc.tensor.matmul(
                        po, ctxT[:, g, st * P:(st + 1) * P],
                        wo_b[:, g, eb * 512:(eb + 1) * 512],
                        start=(g == 0), stop=(g == NG - 1))
                o_sb = work.tile([P, 512], F32, tag="o_sb")
                nc.vector.tensor_copy(o_sb, po)
                nc.sync.dma_start(out_d[st * P:(st + 1) * P, eb * 512:(eb + 1) * 512],
                                  o_sb)

    nc.compile()
    return nc


def _get_nc():
    global _CACHED_NC
    if _CACHED_NC is None:
        _CACHED_NC = build_nc()
    return _CACHED_NC


def _make_in_maps(x, Wq, Wk, Wv, Wo):
    in_maps = []
    for core in range(8):
        b, hg = core // 2, core % 2
        cs = slice(hg * 512, (hg + 1) * 512)
        in_maps.append({
            "x": np.ascontiguousarray(x[b]),
            "wq": np.ascontiguousarray(Wq[:, cs]),
            "wk": np.ascontiguousarray(Wk[:, cs]),
            "wv": np.ascontiguousarray(Wv[:, cs]),
            "wo": np.ascontiguousarray(Wo[cs, :]),
        })
    return in_maps


def run(x, Wq, Wk, Wv, Wo, bo, trace=False):
    nc = _get_nc()
    in_maps = _make_in_maps(x, Wq, Wk, Wv, Wo)
    res = bass_utils.run_bass_kernel_spmd(
        nc, in_maps, core_ids=list(range(8)), trace=trace)
    parts = [r["part"] for r in res.results]
    out = np.empty((4, S, D), dtype=np.float32)
    for b in range(4):
        out[b] = parts[2 * b] + parts[2 * b + 1]
    out += np.asarray(bo, dtype=np.float32)[None, None, :]
    return out, res


def kernel(x, Wq, Wk, Wv, Wo, bo):
    x = np.asarray(x, dtype=np.float32)
    Wq = np.asarray(Wq, dtype=np.float32)
    Wk = np.asarray(Wk, dtype=np.float32)
    Wv = np.asarray(Wv, dtype=np.float32)
    Wo = np.asarray(Wo, dtype=np.float32)
    bo = np.asarray(bo, dtype=np.float32)
    out, _ = run(x, Wq, Wk, Wv, Wo, bo, trace=False)
    return out
